# revision 21
# baseline (speedup 1.0000x reference)
"""Trainium2 Bass kernel for nn_AtomwiseLinear (histogram_binning).

Pipeline (8 NeuronCores, SPMD, no collectives needed):
  host: shard nodes across cores; partition the 32M edge-endpoint entries by
        owning node bucket (hierarchical sharding: core -> 512-node bucket),
        padded to fixed-size buckets; co-locate the other-endpoint's type bit
        with each entry record; ship x transposed/permuted to the hist layout.
  device (per core):
    A) histogram: per bucket, build 2-level one-hots (lo: 32 bins, hi: 16 bins
       x 2 blocks [plain | type-weighted]) with DVE is_equal, contract with PE
       matmuls accumulating in PSUM -> count[v] and A[v] (# type-1 neighbors).
    B) decode: crit = 3*(count>10) + mix, mix from (count, A, own type).
    C) dense: out^T = mask * ((x @ W) / sqrt(30)) via PE matmul + DVE mask.
  host: inverse-permute/transpose device outputs into [1M, 30].
"""

import os
import sys

sys.path.insert(0, "/opt/trn_rl_repo")

import numpy as np
import ml_dtypes

from concourse import bacc, bass, mybir
import concourse.tile as tile
from concourse.bass_utils import run_bass_kernel_spmd

BF16 = ml_dtypes.bfloat16

NCORES = 8
D = 30
WINDOW = 5
DEG_THRESH = 10

MINI = bool(int(os.environ.get("KMINI", "0")))

if MINI:
    LOG_SHARD = 11          # nodes per core
    TPB = 8                 # tiles (of 128 entries) per bucket
else:
    LOG_SHARD = 17
    TPB = int(os.environ.get("KTPB", "132"))

SHARD = 1 << LOG_SHARD
KUNROLL = bool(int(os.environ.get("KUNROLL", "0")))
LO = 32                     # low-bin count (rhs one-hot width)
HI = 16                     # high-bin count (lhsT block width)
BUCKET = LO * HI            # 512 nodes per bucket
NBUCK = SHARD // BUCKET     # buckets per core
CAP = TPB * 128             # padded entries per bucket
NTD = min(512, NBUCK * HI)  # dense-phase node tile (psum free-dim)
CW = NBUCK * LO             # hist free width (columns per hi-chunk)

F32 = mybir.dt.float32
BF = mybir.dt.bfloat16
U8 = mybir.dt.uint8
KU8 = bool(int(os.environ.get("KU8", "1")))
OH = mybir.dt.float8e4 if KU8 else BF
IOTA_NPDT = np.uint8 if KU8 else BF16


def _host_prep(x, W, edge_index, atom_types):
    """Shard + bucket inputs. Returns (in_maps, col_perm, n_real)."""
    n = x.shape[0]
    e0 = np.asarray(edge_index[0], dtype=np.int32)
    e1 = np.asarray(edge_index[1], dtype=np.int32)
    t8 = np.asarray(atom_types, dtype=np.uint8)

    # entry stream: each edge contributes (node=src, w=t[dst]) and (node=dst, w=t[src])
    nodes = np.concatenate([e0, e1])
    wbit = np.concatenate([t8[e1], t8[e0]])

    nbuck_total = NCORES * NBUCK
    gb = (nodes >> 9).astype(np.int16)  # global bucket id (BUCKET=512)
    counts = np.bincount(gb, minlength=nbuck_total)
    if counts.max() > CAP:
        raise RuntimeError(f"bucket overflow: {counts.max()} > {CAP}")
    order = np.argsort(gb, kind="stable")
    gbs = gb[order].astype(np.int64)
    sn = nodes[order]
    sw = wbit[order]
    starts = np.zeros(nbuck_total, dtype=np.int64)
    np.cumsum(counts[:-1], out=starts[1:])
    within = np.arange(nodes.shape[0], dtype=np.int64) - starts[gbs]

    streams = np.full((nbuck_total, 128, 3 * TPB), 255, dtype=np.uint8)
    p = within & 127
    t = within >> 7
    flat = streams.reshape(-1)
    base = gbs * (128 * 3 * TPB) + p * (3 * TPB)
    hi_f = ((sn >> 5) & (HI - 1)).astype(np.uint8)
    flat[base + t] = (sn & (LO - 1)).astype(np.uint8)
    flat[base + TPB + t] = hi_f
    flat[base + 2 * TPB + t] = hi_f + 32 - 32 * sw.astype(np.uint8)

    # dense-layout permutation: local node L -> hist column order
    L = np.arange(SHARD, dtype=np.int64)
    col = (L & (LO - 1)) * (NBUCK * HI) + (L >> 9) * HI + ((L >> 5) & (HI - 1))
    inv = np.empty(SHARD, dtype=np.int64)
    inv[col] = L  # node at flat hist position j is inv[j]

    iota_lo = np.ascontiguousarray(np.broadcast_to(
        np.tile(np.arange(LO, dtype=np.float32), TPB), (128, TPB * LO)
    ).astype(IOTA_NPDT))
    iota_hi = np.ascontiguousarray(np.broadcast_to(
        np.tile(np.arange(HI, dtype=np.float32), TPB), (128, TPB * HI)
    ).astype(IOTA_NPDT))
    d5 = (np.arange(D, dtype=np.float32) // WINDOW).reshape(D, 1)
    wmat = np.asarray(W, dtype=np.float32)

    in_maps = []
    n_real = []
    for c in range(NCORES):
        lo_g = c * SHARD
        hi_g = min(n, (c + 1) * SHARD)
        nc_real = max(0, hi_g - lo_g)
        n_real.append(nc_real)
        xs = np.zeros((SHARD, D), dtype=np.float32)
        ts = np.zeros(SHARD, dtype=np.float32)
        if nc_real > 0:
            xs[:nc_real] = x[lo_g:hi_g]
            ts[:nc_real] = t8[lo_g:hi_g]
        xt = np.ascontiguousarray(xs[inv].T)          # [D, SHARD] in hist order
        th = np.ascontiguousarray(ts[inv]).reshape(LO, NBUCK * HI)
        in_maps.append(
            {
                "streams": streams[c * NBUCK : (c + 1) * NBUCK],
                "xt": xt,
                "th": th,
                "wmat": wmat,
                "iota_lo": iota_lo,
                "iota_hi": iota_hi,
                "d5": d5,
            }
        )
    return in_maps, col, n_real


def build_nc():
    nc = bacc.Bacc("TRN2", target_bir_lowering=False, debug=False, num_devices=NCORES)
    streams_d = nc.dram_tensor("streams", [NBUCK, 128, 3 * TPB], U8, kind="ExternalInput")
    xt_d = nc.dram_tensor("xt", [D, SHARD], F32, kind="ExternalInput")
    th_d = nc.dram_tensor("th", [LO, NBUCK * HI], F32, kind="ExternalInput")
    wmat_d = nc.dram_tensor("wmat", [D, D], F32, kind="ExternalInput")
    IDT = U8 if KU8 else BF
    iota_lo_d = nc.dram_tensor("iota_lo", [128, TPB * LO], IDT, kind="ExternalInput")
    iota_hi_d = nc.dram_tensor("iota_hi", [128, TPB * HI], IDT, kind="ExternalInput")
    d5_d = nc.dram_tensor("d5", [D, 1], F32, kind="ExternalInput")
    outt_d = nc.dram_tensor("outt", [D, SHARD], F32, kind="ExternalOutput")
    critd = nc.dram_tensor("crit_bounce", [LO, NBUCK * HI], F32)
    ybounce = nc.dram_tensor("y_bounce", [D, SHARD], F32)

    with tile.TileContext(nc) as tc:
        with tc.tile_pool(name="const", bufs=1) as cpool:
            iota_lo = cpool.tile([128, TPB * LO], IDT)
            iota_hi = cpool.tile([128, TPB * HI], IDT)
            wsc = cpool.tile([D, D], F32)
            d5 = cpool.tile([D, 1], F32)
            th = cpool.tile([LO, NBUCK * HI], F32)
            hist = cpool.tile([LO, NBUCK * 2 * HI], F32)

            nc.sync.dma_start(out=iota_lo[:], in_=iota_lo_d[:])
            nc.sync.dma_start(out=iota_hi[:], in_=iota_hi_d[:])
            nc.sync.dma_start(out=wsc[:], in_=wmat_d[:])
            nc.scalar.mul(out=wsc[:], in_=wsc[:], mul=float(1.0 / np.sqrt(D)))
            nc.sync.dma_start(out=d5[:], in_=d5_d[:])
            nc.sync.dma_start(out=th[:], in_=th_d[:])

            # ---- Phase A: bucketed 2-level one-hot histogram ----
            wpool = tc.alloc_tile_pool(name="work", bufs=3)
            ppool = tc.alloc_tile_pool(name="psum", bufs=4, space="PSUM")
            import contextlib

            def _bucket_iter():
                if KUNROLL:
                    for bb in range(NBUCK):
                        yield bb
                else:
                    with tc.For_i(
                        0, NBUCK, 1,
                        hint_engines=(mybir.EngineType.PE,),
                        staggered_reset=bool(int(os.environ.get("KSTAG", "0"))),
                    ) as bb:
                        yield bb

            for b in _bucket_iter():
                raw = wpool.tile([128, 3 * TPB], U8, tag="raw")
                nc.sync.dma_start(out=raw[:], in_=streams_d[bass.ds(b, 1), :, :])
                if KU8:
                    lob = raw[:, 0:TPB]
                    hib = raw[:, TPB : 2 * TPB]
                    hi2 = raw[:, 2 * TPB : 3 * TPB]
                else:
                    lobt = wpool.tile([128, TPB], BF, tag="lob")
                    hibt = wpool.tile([128, TPB], BF, tag="hib")
                    hi2t = wpool.tile([128, TPB], BF, tag="hi2")
                    nc.scalar.copy(out=lobt[:], in_=raw[:, 0:TPB])
                    nc.scalar.copy(out=hibt[:], in_=raw[:, TPB : 2 * TPB])
                    nc.scalar.copy(out=hi2t[:], in_=raw[:, 2 * TPB : 3 * TPB])
                    lob, hib, hi2 = lobt[:], hibt[:], hi2t[:]

                ohlo = wpool.tile([128, TPB * LO], OH, tag="ohlo")
                blk = wpool.tile([128, TPB * 2 * HI], OH, tag="blk")
                ohlo3 = ohlo[:].rearrange("p (t n) -> p t n", n=LO)
                ilo3 = iota_lo[:].rearrange("p (t n) -> p t n", n=LO)
                lob3 = lob.to_broadcast([128, TPB, LO])
                nc.vector.tensor_tensor(
                    out=ohlo3, in0=ilo3, in1=lob3, op=mybir.AluOpType.is_equal
                )
                blk3 = blk[:].rearrange("p (t m) -> p t m", m=2 * HI)
                ihi3 = iota_hi[:].rearrange("p (t n) -> p t n", n=HI)
                hib3 = hib.to_broadcast([128, TPB, HI])
                hi23 = hi2.to_broadcast([128, TPB, HI])
                nc.vector.tensor_tensor(
                    out=blk3[:, :, 0:HI], in0=ihi3, in1=hib3,
                    op=mybir.AluOpType.is_equal,
                )
                nc.vector.tensor_tensor(
                    out=blk3[:, :, HI : 2 * HI], in0=ihi3, in1=hi23,
                    op=mybir.AluOpType.is_equal,
                )

                # fused dense chunk(s) for this iteration:
                # y[:, c0:c0+NTD] = (x@W)/sqrt(D), hidden under the DVE one-hots
                cpb = (SHARD // NTD) // NBUCK
                for k in range(cpb):
                    xt_t = wpool.tile([D, NTD], F32, tag="xt1")
                    nc.sync.dma_start(
                        out=xt_t[:],
                        in_=xt_d[:, bass.ds(b * (cpb * NTD) + k * NTD, NTD)],
                    )
                    ps2d = ppool.tile([D, NTD], F32, tag="ps2d")
                    nc.tensor.matmul(ps2d[:], lhsT=wsc[:], rhs=xt_t[:], start=True, stop=True)
                    yt = wpool.tile([D, NTD], F32, tag="yt1")
                    nc.scalar.copy(out=yt[:], in_=ps2d[:])
                    nc.sync.dma_start(
                        out=ybounce[:, bass.ds(b * (cpb * NTD) + k * NTD, NTD)], in_=yt[:]
                    )

                ps = ppool.tile([LO, 2 * HI], F32, tag="ps")
                for t in range(TPB):
                    nc.tensor.matmul(
                        ps[:],
                        lhsT=ohlo[:, t * LO : (t + 1) * LO],
                        rhs=blk[:, t * 2 * HI : (t + 1) * 2 * HI],
                        start=(t == 0),
                        stop=(t == TPB - 1),
                    )
                nc.scalar.copy(out=hist[:, bass.ds(b * 2 * HI, 2 * HI)], in_=ps[:])

            wpool.release()
            ppool.release()

            # ---- Phase B: decode crit = 3*(count>10) + mix ----
            hist3 = hist[:].rearrange("p (b u) -> p b u", u=2 * HI)
            cnt = hist3[:, :, 0:HI]
            aa = hist3[:, :, HI : 2 * HI]
            BW = NBUCK * HI
            ta = cpool.tile([LO, BW], F32)
            tb = cpool.tile([LO, BW], F32)
            crit = cpool.tile([LO, BW], F32)
            AL = mybir.AluOpType

            def v3(t):
                return t[:].rearrange("p (b u) -> p b u", u=HI)

            nc.vector.tensor_scalar(out=v3(ta), in0=aa, scalar1=0.0, scalar2=None, op0=AL.is_equal)
            nc.vector.tensor_scalar(out=tb[:], in0=th[:], scalar1=-1.0, scalar2=1.0, op0=AL.mult, op1=AL.add)
            nc.vector.tensor_tensor(out=ta[:], in0=ta[:], in1=tb[:], op=AL.mult)
            nc.vector.tensor_tensor(out=v3(tb), in0=aa, in1=cnt, op=AL.is_equal)
            nc.vector.scalar_tensor_tensor(out=tb[:], in0=tb[:], scalar=2.0, in1=th[:], op0=AL.mult, op1=AL.mult)
            nc.vector.tensor_tensor(out=ta[:], in0=ta[:], in1=tb[:], op=AL.add)
            nc.vector.tensor_scalar(out=v3(tb), in0=cnt, scalar1=0.0, scalar2=None, op0=AL.is_gt)
            nc.vector.tensor_tensor(out=ta[:], in0=ta[:], in1=tb[:], op=AL.mult)
            nc.vector.tensor_scalar(out=v3(tb), in0=cnt, scalar1=float(DEG_THRESH) + 0.5, scalar2=None, op0=AL.is_gt)
            nc.vector.scalar_tensor_tensor(out=crit[:], in0=tb[:], scalar=3.0, in1=ta[:], op0=AL.mult, op1=AL.add)

            # ---- Phase C: dense (x @ W) * scale * mask ----
            dpool = tc.alloc_tile_pool(name="dense", bufs=3)
            nc.sync.dma_start(out=critd[:], in_=crit[:])
            CWD = NBUCK * HI
            for h in range(LO):
                critb = dpool.tile([D, CWD], F32, tag="critb")
                nc.sync.dma_start(out=critb[:], in_=critd[h : h + 1, :].to_broadcast([D, CWD]))
                for j in range(CWD // NTD):
                    c0 = h * CWD + j * NTD
                    yt2 = dpool.tile([D, NTD], F32, tag="yt2")
                    nc.sync.dma_start(out=yt2[:], in_=ybounce[:, c0 : c0 + NTD])
                    mt = dpool.tile([D, NTD], F32, tag="mt")
                    nc.vector.scalar_tensor_tensor(
                        out=mt[:],
                        in0=critb[:, j * NTD : (j + 1) * NTD],
                        scalar=d5[:],
                        in1=yt2[:],
                        op0=AL.is_equal,
                        op1=AL.mult,
                    )
                    nc.sync.dma_start(out=outt_d[:, c0 : c0 + NTD], in_=mt[:])
            dpool.release()

    nc.compile()
    return nc


def _assemble(results, col, n_real, dtype):
    n = sum(n_real)
    out = np.empty((n, D), dtype=dtype)
    for c in range(NCORES):
        nr = n_real[c]
        if nr == 0:
            continue
        outt = results[c]["outt"]  # [D, SHARD] in hist-column order
        out[c * SHARD : c * SHARD + nr] = outt[:, col[:nr]].T
    return out


def kernel(x, W, edge_index, atom_types):
    x = np.asarray(x)
    in_maps, col, n_real = _host_prep(x, W, edge_index, atom_types)
    nc = build_nc()
    res = run_bass_kernel_spmd(nc, in_maps, list(range(NCORES)))
    return _assemble(res.results, col, n_real, x.dtype)
